# revision 28
# baseline (speedup 1.0000x reference)
"""Complementary gray-code structured-light decoder on 8 Trainium2 NeuronCores.

kernel(images: [24, 2048, 2448] f32) -> [2048, 2448, 2] f32

Sharding: H rows split across 8 cores (256 rows each), data-parallel. Per core
the 256x2448 slab is processed as 8 tiles of [128 rows x 612 cols].

Math (per pixel, direction d in {col,row}; frames 4d..4d+3 = phase steps,
frames 8+8d..15+8d = gray codes):
  c = i0 - i2 ; s = i1 - i3
  s8  = sum of the 8 phase frames  (thr = s8/8)
  b_i = (8*gc_i > s8) ; x_i = b_0^..^b_i          (gray decode, fp8 planes)
  At  = atan(s / (|c|+1e-35))   (ACT Ln/Exp reciprocal + Arctan)
  sgc = sign(c), S' = 16/(2pi)
  out = mask * ( sum_i 2^(10-i)*x_i - S'*(At*sgc) - 4*sgc + 8*(x7*sgc) + 4 )
  mask = (s^2+c^2 > T_EFF) in either direction
(equals the reference unwrap with e = x7 XOR (c>=0).)

v6 engine layout (driven by NTFF measurements):
 - DVE 2-src ops run at ~half rate; the XOR cascade runs as uint32
   bitwise XOR over packed fp8 bytes (4 pixels/lane-element).
 - The gray compare is one full-contiguous STT writing all 16 fp8
   planes; all atan/mask-path tiles use plane pitch PX=616 so every op
   is contiguous (pad lanes compute garbage, never read by the PE).
 - PE: 4 DoubleRow fp8 paired matmuls + bf16 planes AtS (w=-S'),
   sgc (w=-4), x7s (w=8) per PSUM quadrant; PSUM evac + (+4) + mask
   multiply fused into one DVE STT from PSUM.
 - GPSIMD only gets ops on compute-produced tiles (U, Q).
"""
import numpy as np

import concourse.bass as bass
import concourse.mybir as mybir
import concourse.tile as tile
from concourse.ap import AP
from concourse.vector_clock import ScopedClock
from concourse.bass_utils import run_bass_kernel_spmd


def _apv(tile_obj, dims, offset):
    """Custom strided free-dim view of a [128, N] tile: dims is a list of
    (elem_stride, size) free dims; offset in elements."""
    base = tile_obj[:, :]
    part = base.ap[0]
    return AP(base.tensor, offset, [list(part)] + [[s, n] for s, n in dims])

# ---------------- constants ----------------
H, W = 2048, 2448
NFRAMES = 24
NCORES = 8
ROWS_PER_CORE = H // NCORES          # 256
F = 612                              # tile free width; W = 4*F
FH = F // 2                          # 306, PSUM-bank-sized half
F2 = 2 * F
F4 = 4 * F
F8 = 8 * F
PX = 616                             # plane pitch (2*PX % 16 == 0)
PX2 = 2 * PX
NT_R = ROWS_PER_CORE // 128          # 2
NT_C = W // F                        # 4

T_EFF = 0.010000010952353477         # (q > T_EFF) == (0.5*sqrt(q) > 0.05f)
S_PRIME = float(np.float32(16.0 / (2.0 * np.float64(np.pi))))

f32 = mybir.dt.float32
bf16 = mybir.dt.bfloat16
fp8 = mybir.dt.float8e5
u32 = mybir.dt.uint32
OP = mybir.AluOpType
AF = mybir.ActivationFunctionType
PM = mybir.MatmulPerfMode

_ctr = [0]


def _sanitize_waits(nc):
    """This walrus build rejects instructions carrying >1 sync wait. Move
    excess waits onto fresh same-engine NOPs inserted just before."""
    for f in nc.m.functions:
        for bb in f.blocks:
            il = bb.instructions
            i = 0
            while i < len(il):
                ins = il[i]
                si = getattr(ins, "sync_info", None)
                waits = list(si.on_wait) if si is not None and si.on_wait else []
                if len(waits) > 1:
                    si.on_wait = [waits[-1]]
                    ins.sync_info = si
                    for w in waits[:-1]:
                        _ctr[0] += 1
                        n = mybir.InstNoOp(name=f"waitsplit_{_ctr[0]}")
                        n.engine = ins.engine
                        n.sync_info = mybir.SyncInfo(on_wait=[w], on_update=[])
                        il.insert(i, n)
                        i += 1
                i += 1


class _SafeTileContext(tile.TileContext):
    """TileContext whose exit drain splits its sem waits across SP NOPs
    (the drain is emitted inside __exit__, before _sanitize_waits can run)."""

    def _drain_and_barrier(self, tick_clock, wait_clock):
        nop_inst = self.nc.sync.nop()
        wait_clock.add_sem_waits(
            nop_inst.ins, ScopedClock({None: tick_clock.global_clock})
        )
        si = nop_inst.ins.sync_info
        waits = list(si.on_wait) if si is not None else []
        if len(waits) > 1:
            si.on_wait = waits[:1]
            nop_inst.ins.sync_info = si
            for w in waits[1:]:
                n2 = self.nc.sync.nop()
                n2.ins.sync_info = mybir.SyncInfo(on_wait=[w], on_update=[])
        self.nc.sync.drain()

        self.nc.all_engine_barrier()
        assert self.sems is not None
        popped = self.nc._tile_sem_poison_stack.pop()
        assert popped is self._sem_poison
        self.nc.clear_and_free_semaphores(list(self.sems.allocated().values()))
        self.nc.all_engine_barrier()


def _build_program(sanitize=True):
    import contextlib

    nc = bass.Bass("TRN2", target_bir_lowering=False, debug=False)
    img = nc.dram_tensor("img", [NFRAMES, ROWS_PER_CORE, W], f32, kind="ExternalInput")
    wdr = nc.dram_tensor("wdr", [4, 128, 2, 128], fp8, kind="ExternalInput")
    wbf = nc.dram_tensor("wbf", [3, 128, 128], bf16, kind="ExternalInput")
    out = nc.dram_tensor("out", [ROWS_PER_CORE, W, 2], f32, kind="ExternalOutput")

    with _SafeTileContext(nc) as tc, contextlib.ExitStack() as ctx:
        wpool = ctx.enter_context(tc.tile_pool(name="wpool", bufs=1))
        ps_in = ctx.enter_context(tc.tile_pool(name="ps_in", bufs=2))
        gc_in = ctx.enter_context(tc.tile_pool(name="gc_in", bufs=2))
        bpool = ctx.enter_context(tc.tile_pool(name="bpool", bufs=2))
        sb = ctx.enter_context(tc.tile_pool(name="sb", bufs=1))
        tailp = ctx.enter_context(tc.tile_pool(name="tailp", bufs=2))
        outp = ctx.enter_context(tc.tile_pool(name="outp", bufs=2))
        psum = ctx.enter_context(tc.tile_pool(name="psum", bufs=2, space="PSUM"))

        b35 = wpool.tile([128, 1], f32, tag="b35")
        nc.vector.memset(b35[:, :], 1e-35)
        wdr_t = wpool.tile([128, 4 * 2 * 128], fp8, tag="wdr_t")
        for k in range(4):
            nc.sync.dma_start(
                out=wdr_t[:, k * 256:(k + 1) * 256].rearrange("p (m c) -> p m c", m=2),
                in_=wdr[k, :, :, :])
        wbf_t = wpool.tile([128, 3 * 128], bf16, tag="wbf_t")
        for k in range(3):
            nc.sync.dma_start(
                out=wbf_t[:, k * 128:(k + 1) * 128], in_=wbf[k, :, :])
        w_ats = wbf_t[:, 0:128]      # diag(-S')
        w_sgc = wbf_t[:, 128:256]    # diag(-4)
        w_x7s = wbf_t[:, 256:384]    # diag(8)

        for rb in range(NT_R):
            r0 = rb * 128
            for cb in range(NT_C):
                c0 = cb * F
                # ---------------- loads (+4 pad tail for padded views) -----
                Xps = ps_in.tile([128, F8 + 4], f32, tag="xps")
                nc.sync.dma_start(
                    out=Xps[:, 0:F8].rearrange("p (f x) -> p f x", f=8),
                    in_=img[0:8, r0:r0 + 128, c0:c0 + F].rearrange("f p x -> p f x"),
                )
                Xgc = gc_in.tile([128, 16 * F + 4], f32, tag="xgc")
                nc.sync.dma_start(
                    out=Xgc[:, 0:16 * F].rearrange("p (f x) -> p f x", f=16),
                    in_=img[8:24, r0:r0 + 128, c0:c0 + F].rearrange("f p x -> p f x"),
                )

                # ---------------- c/s (DVE, one op, PX-pitch) --------------
                # cs = [c_col | c_row | s_col | s_row], pitch PX, pads junk
                nc.vector.memset(Xps[:, F8:F8 + 4], 0.0)
                nc.vector.memset(Xgc[:, 16 * F:16 * F + 4], 0.0)
                cs = sb.tile([128, 4 * PX], f32, tag="cs")
                for g in range(2):
                    nc.vector.tensor_tensor(
                        _apv(cs, [(PX2, 2), (1, PX)], g * PX),
                        _apv(Xps, [(F, 2), (1, PX)], g * F4),
                        _apv(Xps, [(F, 2), (1, PX)], g * F4 + F2),
                        OP.subtract)
                cB = cs[:, 0:PX2]          # [c_col | c_row], contiguous
                sB = cs[:, PX2:4 * PX]     # [s_col | s_row], contiguous

                # ------- s8 = sum of 8 frames (t1/s8 DVE, t2 GPSIMD) -------
                T1 = sb.tile([128, F4 + 8], f32, tag="t1")
                nc.vector.tensor_tensor(
                    T1[:, 0:F4], Xps[:, 0:F4], Xps[:, F4:F8], OP.add)
                nc.gpsimd.tensor_tensor(
                    T1[:, 0:F2], T1[:, 0:F2], T1[:, F2:F4], OP.add)
                S8 = sb.tile([128, PX], f32, tag="s8")
                nc.vector.memset(S8[:, F:PX], 0.0)
                nc.vector.tensor_tensor(
                    S8[:, 0:F], T1[:, 0:F], T1[:, F:F2], OP.add)

                # ------- gray compare -> fp8 planes (DVE) ------------------
                # B plane (bit i, dir d) at offset (2i+d)*PX; b_i = (8*gc>S8)
                B = bpool.tile([128, 16 * PX], fp8, tag="B")
                for dd in range(2):
                    nc.vector.scalar_tensor_tensor(
                        _apv(B, [(PX2, 8), (1, PX)], dd * PX),
                        _apv(Xgc, [(F, 8), (1, PX)], dd * 8 * F),
                        8.0,
                        _apv(S8, [(0, 8), (1, PX)], 0),
                        OP.mult, OP.is_gt,
                    )

                # ---------------- atan path (ACT ln/exp reciprocal) --------
                ac = sb.tile([128, PX2], f32, tag="ac")
                nc.scalar.activation(ac[:, :], cB, AF.Abs, bias=0.0, scale=1.0)
                lc = sb.tile([128, PX2], f32, tag="lc")
                nc.scalar.activation(lc[:, :], ac[:, :], AF.Ln, bias=b35[:, :], scale=1.0)
                # ec reuses ac (|c| dead after Ln)
                nc.scalar.activation(ac[:, :], lc[:, :], AF.Exp, bias=0.0, scale=-1.0)
                sgc = sb.tile([128, PX2], bf16, tag="sgc")
                nc.scalar.activation(sgc[:, :], cB, AF.Sign, bias=0.0, scale=1.0)
                U = sb.tile([128, PX2], f32, tag="u")
                nc.gpsimd.tensor_tensor(U[:, :], sB, ac[:, :], OP.mult)
                ATB = sb.tile([128, PX2], bf16, tag="atb")
                nc.scalar.activation(ATB[:, :], U[:, :], AF.Arctan, bias=0.0, scale=1.0)

                # -------- XOR cascade (in place over B, uint32 packed) -----
                PU = PX // 2  # u32 elems per plane-pair
                Bu = B[:, :].bitcast(u32)
                for i in range(1, 8):
                    nc.vector.tensor_tensor(
                        Bu[:, i * PU:(i + 1) * PU],
                        Bu[:, (i - 1) * PU:i * PU],
                        Bu[:, i * PU:(i + 1) * PU],
                        OP.bitwise_xor)

                # AtS = At*sgc, x7s = x7*sgc  (both bf16, contiguous).
                # x7 fp8 -> bf16 via ACT copy first (fp8-src TT is 1x rate);
                # the bf16 x7 copy reuses ATB once AtS has consumed it.
                AtS = tailp.tile([128, PX2], bf16, tag="ats")
                nc.vector.tensor_tensor(AtS[:, :], ATB[:, :], sgc[:, :], OP.mult)
                nc.scalar.activation(
                    ATB[:, :], B[:, 14 * PX:16 * PX], AF.Copy, bias=0.0, scale=1.0)
                x7s = tailp.tile([128, PX2], bf16, tag="x7s")
                nc.vector.tensor_tensor(
                    x7s[:, :], ATB[:, :], sgc[:, :], OP.mult)

                # ---------------- mask ----------------
                # squares reuse lc (dead after Exp) and U (dead after Arctan)
                nc.scalar.activation(lc[:, :], cB, AF.Square, bias=0.0, scale=1.0)
                nc.scalar.activation(U[:, :], sB, AF.Square, bias=0.0, scale=1.0)
                # q reuses ac (ec dead after U-mult)
                nc.gpsimd.tensor_tensor(ac[:, :], lc[:, :], U[:, :], OP.add)
                bit_r = tailp.tile([128, PX], f32, tag="bit_r")
                nc.vector.tensor_scalar(
                    bit_r[:, :], ac[:, PX:PX2], T_EFF, None, OP.is_gt)
                maskb = tailp.tile([128, PX], f32, tag="maskb")
                nc.vector.scalar_tensor_tensor(
                    maskb[:, :], ac[:, 0:PX], T_EFF, bit_r[:, :], OP.is_gt, OP.max)

                # ---------------- PE combine ----------------
                Bp = B[:, :].rearrange("p (k m d x) -> p k m d x", k=4, m=2, d=2)
                tps = [
                    psum.tile([128, FH], f32, tag=f"t_{d}{h}", name=f"t_{d}{h}")
                    for d in range(2) for h in range(2)
                ]
                for d in range(2):
                    for h in range(2):
                        pt = tps[d * 2 + h]
                        for k in range(4):
                            nc.tensor.matmul(
                                pt[:, :],
                                wdr_t[:, k * 256:(k + 1) * 256]
                                .rearrange("p (m c) -> p m c", m=2),
                                Bp[:, k, :, d, h * FH:(h + 1) * FH],
                                start=(k == 0), stop=False,
                                perf_mode=PM.DoubleRow)
                        for wsl, plane in ((w_ats, AtS), (w_sgc, sgc)):
                            nc.tensor.matmul(
                                pt[:, :], wsl,
                                plane[:, d * PX + h * FH: d * PX + (h + 1) * FH],
                                start=False, stop=False)
                        nc.tensor.matmul(
                            pt[:, :], w_x7s,
                            x7s[:, d * PX + h * FH: d * PX + (h + 1) * FH],
                            start=False, stop=True)

                # ------- fused evac: out = (psum + 4) * mask  (DVE) --------
                o_t = outp.tile([128, F2], f32, tag="o_t")
                ov = o_t[:, :].rearrange("p (x two) -> p two x", two=2)
                for d in range(2):
                    for h in range(2):
                        nc.vector.scalar_tensor_tensor(
                            ov[:, d, h * FH:(h + 1) * FH],
                            tps[d * 2 + h][:, :], 4.0,
                            maskb[:, h * FH:(h + 1) * FH],
                            OP.add, OP.mult)
                nc.sync.dma_start(
                    out=out[r0:r0 + 128, c0:c0 + F, :].rearrange("p x two -> p (x two)"),
                    in_=o_t[:, :],
                )

    if sanitize:
        _sanitize_waits(nc)
    return nc


def _weights():
    import ml_dtypes
    I = np.eye(128, dtype=np.float32)
    wdr = np.zeros((4, 128, 2, 128), dtype=np.float32)
    for k in range(4):
        for m in range(2):
            wdr[k, :, m, :] = (2.0 ** (10 - (2 * k + m))) * I
    wbf = np.stack([-S_PRIME * I, -4.0 * I, 8.0 * I]).astype(ml_dtypes.bfloat16)
    return wdr.astype(ml_dtypes.float8_e5m2), wbf


_CACHE = {}


def _in_maps(images):
    wdr, wbf = _weights()
    maps = []
    for core in range(NCORES):
        r0 = core * ROWS_PER_CORE
        maps.append({
            "img": np.ascontiguousarray(images[:, r0:r0 + ROWS_PER_CORE, :]),
            "wdr": wdr,
            "wbf": wbf,
        })
    return maps


def kernel(images: np.ndarray) -> np.ndarray:
    images = np.ascontiguousarray(np.asarray(images, dtype=np.float32))
    assert images.shape == (NFRAMES, H, W), images.shape
    if "nc" not in _CACHE:
        _CACHE["nc"] = _build_program()
    res = run_bass_kernel_spmd(_CACHE["nc"], _in_maps(images), core_ids=list(range(NCORES)))
    out = np.empty((H, W, 2), dtype=np.float32)
    for core in range(NCORES):
        r0 = core * ROWS_PER_CORE
        out[r0:r0 + ROWS_PER_CORE] = res.results[core]["out"]
    return out


def timed_run(images: np.ndarray):
    """Run once with NTFF tracing; returns max per-core exec_time_ns or None."""
    images = np.ascontiguousarray(np.asarray(images, dtype=np.float32))
    if "nc" not in _CACHE:
        _CACHE["nc"] = _build_program()
    try:
        res = run_bass_kernel_spmd(
            _CACHE["nc"], _in_maps(images), core_ids=list(range(NCORES)),
            trace=True, trace_cores=[0],
        )
        return res.exec_time_ns
    except Exception as exc:
        print(f"timed_run: trace failed ({exc})")
        return None


if __name__ == "__main__":
    rng = np.random.default_rng(0)
    imgs = rng.random((NFRAMES, H, W), dtype=np.float32)
    o = kernel(imgs)
    print("ran:", o.shape, o.dtype, float(np.abs(o).max()))


# revision 29
# speedup vs baseline: 1.2534x; 1.2534x over previous
"""Complementary gray-code structured-light decoder on 8 Trainium2 NeuronCores.

kernel(images: [24, 2048, 2448] f32) -> [2048, 2448, 2] f32

Sharding: H rows split across 8 cores (256 rows each), data-parallel. Per core
the 256x2448 slab is processed as 8 tiles of [128 rows x 612 cols].

Math (per pixel, direction d in {col,row}; frames 4d..4d+3 = phase steps,
frames 8+8d..15+8d = gray codes):
  c = i0 - i2 ; s = i1 - i3
  s8  = sum of the 8 phase frames  (thr = s8/8)
  b_i = (8*gc_i > s8) ; x_i = b_0^..^b_i          (gray decode, fp8 planes)
  At  = atan(s / (|c|+1e-35))   (ACT Ln/Exp reciprocal + Arctan)
  sgc = sign(c), S' = 16/(2pi)
  out = mask * ( sum_i 2^(10-i)*x_i - S'*(At*sgc) - 4*sgc + 8*(x7*sgc) + 4 )
  mask = (s^2+c^2 > T_EFF) in either direction
(equals the reference unwrap with e = x7 XOR (c>=0).)

v6 engine layout (driven by NTFF measurements):
 - DVE 2-src ops run at ~half rate; the XOR cascade runs as uint32
   bitwise XOR over packed fp8 bytes (4 pixels/lane-element).
 - The gray compare is one full-contiguous STT writing all 16 fp8
   planes; all atan/mask-path tiles use plane pitch PX=616 so every op
   is contiguous (pad lanes compute garbage, never read by the PE).
 - PE: 4 DoubleRow fp8 paired matmuls + bf16 planes AtS (w=-S'),
   sgc (w=-4), x7s (w=8) per PSUM quadrant; PSUM evac + (+4) + mask
   multiply fused into one DVE STT from PSUM.
 - GPSIMD only gets ops on compute-produced tiles (U, Q).
"""
import numpy as np

import concourse.bass as bass
import concourse.mybir as mybir
import concourse.tile as tile
from concourse.ap import AP
from concourse.vector_clock import ScopedClock
from concourse.bass_utils import run_bass_kernel_spmd


def _apv(tile_obj, dims, offset):
    """Custom strided free-dim view of a [128, N] tile: dims is a list of
    (elem_stride, size) free dims; offset in elements."""
    base = tile_obj[:, :]
    part = base.ap[0]
    return AP(base.tensor, offset, [list(part)] + [[s, n] for s, n in dims])

# ---------------- constants ----------------
H, W = 2048, 2448
NFRAMES = 24
NCORES = 8
ROWS_PER_CORE = H // NCORES          # 256
F = 612                              # tile free width; W = 4*F
FH = F // 2                          # 306, PSUM-bank-sized half
F2 = 2 * F
F4 = 4 * F
F8 = 8 * F
PX = 616                             # plane pitch (2*PX % 16 == 0)
PX2 = 2 * PX
NT_R = ROWS_PER_CORE // 128          # 2
NT_C = W // F                        # 4

T_EFF = 0.010000010952353477         # (q > T_EFF) == (0.5*sqrt(q) > 0.05f)
S_PRIME = float(np.float32(16.0 / (2.0 * np.float64(np.pi))))

f32 = mybir.dt.float32
bf16 = mybir.dt.bfloat16
fp8 = mybir.dt.float8e5
u32 = mybir.dt.uint32
OP = mybir.AluOpType
AF = mybir.ActivationFunctionType
PM = mybir.MatmulPerfMode

_ctr = [0]


def _sanitize_waits(nc):
    """This walrus build rejects instructions carrying >1 sync wait. Move
    excess waits onto fresh same-engine NOPs inserted just before."""
    for f in nc.m.functions:
        for bb in f.blocks:
            il = bb.instructions
            i = 0
            while i < len(il):
                ins = il[i]
                si = getattr(ins, "sync_info", None)
                waits = list(si.on_wait) if si is not None and si.on_wait else []
                if len(waits) > 1:
                    si.on_wait = [waits[-1]]
                    ins.sync_info = si
                    for w in waits[:-1]:
                        _ctr[0] += 1
                        n = mybir.InstNoOp(name=f"waitsplit_{_ctr[0]}")
                        n.engine = ins.engine
                        n.sync_info = mybir.SyncInfo(on_wait=[w], on_update=[])
                        il.insert(i, n)
                        i += 1
                i += 1


class _SafeTileContext(tile.TileContext):
    """TileContext whose exit drain splits its sem waits across SP NOPs
    (the drain is emitted inside __exit__, before _sanitize_waits can run)."""

    def _drain_and_barrier(self, tick_clock, wait_clock):
        nop_inst = self.nc.sync.nop()
        wait_clock.add_sem_waits(
            nop_inst.ins, ScopedClock({None: tick_clock.global_clock})
        )
        si = nop_inst.ins.sync_info
        waits = list(si.on_wait) if si is not None else []
        if len(waits) > 1:
            si.on_wait = waits[:1]
            nop_inst.ins.sync_info = si
            for w in waits[1:]:
                n2 = self.nc.sync.nop()
                n2.ins.sync_info = mybir.SyncInfo(on_wait=[w], on_update=[])
        self.nc.sync.drain()

        self.nc.all_engine_barrier()
        assert self.sems is not None
        popped = self.nc._tile_sem_poison_stack.pop()
        assert popped is self._sem_poison
        self.nc.clear_and_free_semaphores(list(self.sems.allocated().values()))
        self.nc.all_engine_barrier()


def _build_program(sanitize=True):
    import contextlib

    nc = bass.Bass("TRN2", target_bir_lowering=False, debug=False)
    img = nc.dram_tensor("img", [NFRAMES, ROWS_PER_CORE, W], f32, kind="ExternalInput")
    wdr = nc.dram_tensor("wdr", [4, 128, 2, 128], fp8, kind="ExternalInput")
    wbf = nc.dram_tensor("wbf", [3, 128, 128], bf16, kind="ExternalInput")
    out = nc.dram_tensor("out", [ROWS_PER_CORE, W, 2], f32, kind="ExternalOutput")

    with _SafeTileContext(nc) as tc, contextlib.ExitStack() as ctx:
        wpool = ctx.enter_context(tc.tile_pool(name="wpool", bufs=1))
        ps_in = ctx.enter_context(tc.tile_pool(name="ps_in", bufs=2))
        gc_in = ctx.enter_context(tc.tile_pool(name="gc_in", bufs=2))
        bpool = ctx.enter_context(tc.tile_pool(name="bpool", bufs=2))
        sb = ctx.enter_context(tc.tile_pool(name="sb", bufs=1))
        tailp = ctx.enter_context(tc.tile_pool(name="tailp", bufs=2))
        outp = ctx.enter_context(tc.tile_pool(name="outp", bufs=2))
        psum = ctx.enter_context(tc.tile_pool(name="psum", bufs=2, space="PSUM"))

        b35 = wpool.tile([128, 1], f32, tag="b35")
        nc.vector.memset(b35[:, :], 1e-35)
        wdr_t = wpool.tile([128, 4 * 2 * 128], fp8, tag="wdr_t")
        for k in range(4):
            nc.sync.dma_start(
                out=wdr_t[:, k * 256:(k + 1) * 256].rearrange("p (m c) -> p m c", m=2),
                in_=wdr[k, :, :, :])
        wbf_t = wpool.tile([128, 3 * 128], bf16, tag="wbf_t")
        for k in range(3):
            nc.sync.dma_start(
                out=wbf_t[:, k * 128:(k + 1) * 128], in_=wbf[k, :, :])
        w_ats = wbf_t[:, 0:128]      # diag(-S')
        w_sgc = wbf_t[:, 128:256]    # diag(-4)
        w_x7s = wbf_t[:, 256:384]    # diag(8)

        for rb in range(NT_R):
            r0 = rb * 128
            for cb in range(NT_C):
                c0 = cb * F
                # ---------------- loads (+4 pad tail for padded views) -----
                Xps = ps_in.tile([128, F8 + 4], f32, tag="xps")
                nc.sync.dma_start(
                    out=Xps[:, 0:F8].rearrange("p (f x) -> p f x", f=8),
                    in_=img[0:8, r0:r0 + 128, c0:c0 + F].rearrange("f p x -> p f x"),
                )
                Xgc = gc_in.tile([128, 16 * F + 4], f32, tag="xgc")
                nc.sync.dma_start(
                    out=Xgc[:, 0:16 * F].rearrange("p (f x) -> p f x", f=16),
                    in_=img[8:24, r0:r0 + 128, c0:c0 + F].rearrange("f p x -> p f x"),
                )

                # ---------------- c/s (DVE, one op, PX-pitch) --------------
                # cs = [c_col | c_row | s_col | s_row], pitch PX, pads junk
                nc.vector.memset(Xps[:, F8:F8 + 4], 0.0)
                nc.vector.memset(Xgc[:, 16 * F:16 * F + 4], 0.0)
                cs = sb.tile([128, 4 * PX], f32, tag="cs")
                for g in range(2):
                    nc.vector.tensor_tensor(
                        _apv(cs, [(PX2, 2), (1, PX)], g * PX),
                        _apv(Xps, [(F, 2), (1, PX)], g * F4),
                        _apv(Xps, [(F, 2), (1, PX)], g * F4 + F2),
                        OP.subtract)
                cB = cs[:, 0:PX2]          # [c_col | c_row], contiguous
                sB = cs[:, PX2:4 * PX]     # [s_col | s_row], contiguous

                # ------- s8 = sum of 8 frames (t1/s8 DVE, t2 GPSIMD) -------
                T1 = sb.tile([128, F4 + 8], f32, tag="t1")
                nc.vector.tensor_tensor(
                    T1[:, 0:F4], Xps[:, 0:F4], Xps[:, F4:F8], OP.add)
                nc.vector.tensor_tensor(
                    T1[:, 0:F2], T1[:, 0:F2], T1[:, F2:F4], OP.add)
                S8 = sb.tile([128, PX], f32, tag="s8")
                nc.vector.memset(S8[:, F:PX], 0.0)
                nc.vector.tensor_tensor(
                    S8[:, 0:F], T1[:, 0:F], T1[:, F:F2], OP.add)

                # ------- gray compare -> fp8 planes (DVE) ------------------
                # B plane (bit i, dir d) at offset (2i+d)*PX; b_i = (8*gc>S8)
                B = bpool.tile([128, 16 * PX], fp8, tag="B")
                for dd in range(2):
                    nc.vector.scalar_tensor_tensor(
                        _apv(B, [(PX2, 8), (1, PX)], dd * PX),
                        _apv(Xgc, [(F, 8), (1, PX)], dd * 8 * F),
                        8.0,
                        _apv(S8, [(0, 8), (1, PX)], 0),
                        OP.mult, OP.is_gt,
                    )

                # ---------------- atan path (ACT ln/exp reciprocal) --------
                ac = sb.tile([128, PX2], f32, tag="ac")
                nc.scalar.activation(ac[:, :], cB, AF.Abs, bias=0.0, scale=1.0)
                lc = sb.tile([128, PX2], f32, tag="lc")
                nc.scalar.activation(lc[:, :], ac[:, :], AF.Ln, bias=b35[:, :], scale=1.0)
                # ec reuses ac (|c| dead after Ln)
                nc.scalar.activation(ac[:, :], lc[:, :], AF.Exp, bias=0.0, scale=-1.0)
                sgc = sb.tile([128, PX2], bf16, tag="sgc")
                nc.scalar.activation(sgc[:, :], cB, AF.Sign, bias=0.0, scale=1.0)
                U = sb.tile([128, PX2], f32, tag="u")
                nc.gpsimd.tensor_tensor(U[:, :], sB, ac[:, :], OP.mult)
                ATB = sb.tile([128, PX2], bf16, tag="atb")
                nc.scalar.activation(ATB[:, :], U[:, :], AF.Arctan, bias=0.0, scale=1.0)

                # -------- XOR cascade (in place over B, uint32 packed) -----
                PU = PX // 2  # u32 elems per plane-pair
                Bu = B[:, :].bitcast(u32)
                for i in range(1, 8):
                    nc.vector.tensor_tensor(
                        Bu[:, i * PU:(i + 1) * PU],
                        Bu[:, (i - 1) * PU:i * PU],
                        Bu[:, i * PU:(i + 1) * PU],
                        OP.bitwise_xor)

                # AtS = At*sgc, x7s = x7*sgc  (both bf16, contiguous).
                # x7 fp8 -> bf16 via ACT copy first (fp8-src TT is 1x rate);
                # the bf16 x7 copy reuses ATB once AtS has consumed it.
                AtS = tailp.tile([128, PX2], bf16, tag="ats")
                nc.vector.tensor_tensor(AtS[:, :], ATB[:, :], sgc[:, :], OP.mult)
                nc.scalar.activation(
                    ATB[:, :], B[:, 14 * PX:16 * PX], AF.Copy, bias=0.0, scale=1.0)
                x7s = tailp.tile([128, PX2], bf16, tag="x7s")
                nc.vector.tensor_tensor(
                    x7s[:, :], ATB[:, :], sgc[:, :], OP.mult)

                # ---------------- mask ----------------
                # squares reuse lc (dead after Exp) and U (dead after Arctan)
                nc.scalar.activation(lc[:, :], cB, AF.Square, bias=0.0, scale=1.0)
                nc.scalar.activation(U[:, :], sB, AF.Square, bias=0.0, scale=1.0)
                # q reuses ac (ec dead after U-mult)
                nc.gpsimd.tensor_tensor(ac[:, :], lc[:, :], U[:, :], OP.add)
                bit_r = tailp.tile([128, PX], f32, tag="bit_r")
                nc.vector.tensor_scalar(
                    bit_r[:, :], ac[:, PX:PX2], T_EFF, None, OP.is_gt)
                maskb = tailp.tile([128, PX], f32, tag="maskb")
                nc.vector.scalar_tensor_tensor(
                    maskb[:, :], ac[:, 0:PX], T_EFF, bit_r[:, :], OP.is_gt, OP.max)

                # ---------------- PE combine ----------------
                Bp = B[:, :].rearrange("p (k m d x) -> p k m d x", k=4, m=2, d=2)
                tps = [
                    psum.tile([128, FH], f32, tag=f"t_{d}{h}", name=f"t_{d}{h}")
                    for d in range(2) for h in range(2)
                ]
                for d in range(2):
                    for h in range(2):
                        pt = tps[d * 2 + h]
                        for k in range(4):
                            nc.tensor.matmul(
                                pt[:, :],
                                wdr_t[:, k * 256:(k + 1) * 256]
                                .rearrange("p (m c) -> p m c", m=2),
                                Bp[:, k, :, d, h * FH:(h + 1) * FH],
                                start=(k == 0), stop=False,
                                perf_mode=PM.DoubleRow)
                        for wsl, plane in ((w_ats, AtS), (w_sgc, sgc)):
                            nc.tensor.matmul(
                                pt[:, :], wsl,
                                plane[:, d * PX + h * FH: d * PX + (h + 1) * FH],
                                start=False, stop=False)
                        nc.tensor.matmul(
                            pt[:, :], w_x7s,
                            x7s[:, d * PX + h * FH: d * PX + (h + 1) * FH],
                            start=False, stop=True)

                # ------- fused evac: out = (psum + 4) * mask  (DVE) --------
                o_t = outp.tile([128, F2], f32, tag="o_t")
                ov = o_t[:, :].rearrange("p (x two) -> p two x", two=2)
                for d in range(2):
                    for h in range(2):
                        nc.vector.scalar_tensor_tensor(
                            ov[:, d, h * FH:(h + 1) * FH],
                            tps[d * 2 + h][:, :], 4.0,
                            maskb[:, h * FH:(h + 1) * FH],
                            OP.add, OP.mult)
                nc.sync.dma_start(
                    out=out[r0:r0 + 128, c0:c0 + F, :].rearrange("p x two -> p (x two)"),
                    in_=o_t[:, :],
                )

    if sanitize:
        _sanitize_waits(nc)
    return nc


def _weights():
    import ml_dtypes
    I = np.eye(128, dtype=np.float32)
    wdr = np.zeros((4, 128, 2, 128), dtype=np.float32)
    for k in range(4):
        for m in range(2):
            wdr[k, :, m, :] = (2.0 ** (10 - (2 * k + m))) * I
    wbf = np.stack([-S_PRIME * I, -4.0 * I, 8.0 * I]).astype(ml_dtypes.bfloat16)
    return wdr.astype(ml_dtypes.float8_e5m2), wbf


_CACHE = {}


def _in_maps(images):
    wdr, wbf = _weights()
    maps = []
    for core in range(NCORES):
        r0 = core * ROWS_PER_CORE
        maps.append({
            "img": np.ascontiguousarray(images[:, r0:r0 + ROWS_PER_CORE, :]),
            "wdr": wdr,
            "wbf": wbf,
        })
    return maps


def kernel(images: np.ndarray) -> np.ndarray:
    images = np.ascontiguousarray(np.asarray(images, dtype=np.float32))
    assert images.shape == (NFRAMES, H, W), images.shape
    if "nc" not in _CACHE:
        _CACHE["nc"] = _build_program()
    res = run_bass_kernel_spmd(_CACHE["nc"], _in_maps(images), core_ids=list(range(NCORES)))
    out = np.empty((H, W, 2), dtype=np.float32)
    for core in range(NCORES):
        r0 = core * ROWS_PER_CORE
        out[r0:r0 + ROWS_PER_CORE] = res.results[core]["out"]
    return out


def timed_run(images: np.ndarray):
    """Run once with NTFF tracing; returns max per-core exec_time_ns or None."""
    images = np.ascontiguousarray(np.asarray(images, dtype=np.float32))
    if "nc" not in _CACHE:
        _CACHE["nc"] = _build_program()
    try:
        res = run_bass_kernel_spmd(
            _CACHE["nc"], _in_maps(images), core_ids=list(range(NCORES)),
            trace=True, trace_cores=[0],
        )
        return res.exec_time_ns
    except Exception as exc:
        print(f"timed_run: trace failed ({exc})")
        return None


if __name__ == "__main__":
    rng = np.random.default_rng(0)
    imgs = rng.random((NFRAMES, H, W), dtype=np.float32)
    o = kernel(imgs)
    print("ran:", o.shape, o.dtype, float(np.abs(o).max()))
